# revision 5
# baseline (speedup 1.0000x reference)
"""Trainium2 Bass kernel for nn_All_Graph_Net (2-layer pool-SAGE on two graphs).

Strategy (8 NeuronCores, SPMD):
  - Nodes sharded by destination across cores; edges partitioned by dst.
  - Per hop: each core computes m = relu(h @ Wp.T + bp) for its node shard,
    AllGather's m into a per-core DRAM table (in degree-sorted "permuted"
    id space), then per block of 128 dst nodes dma_gathers neighbor rows
    and reduces with a strided DVE max.  Output rows are scattered back to
    natural order with an indirect DMA.
  - All padding points at dedicated zero rows of the m tables; since
    m = relu(...) >= 0, max with 0 reproduces segment_max + zero-degree
    semantics exactly.
  - PPI's permuted table (40962 rows) exceeds dma_gather's int16 index
    range, so it is gathered through two strided row-pair views (even/odd
    ids, stride 1024B), each addressable with indices <= 20480.
"""

import numpy as np

import concourse.bass as bass
import concourse.bacc as bacc
import concourse.mybir as mybir
import concourse.tile as tile
from concourse import bass_utils
from concourse.masks import make_identity

NC = 8
D = 128
P = 128

N_DR, N_P = 20000, 40000
R_D, R_P = N_DR // NC, N_P // NC          # 2500 / 5000 rows per core
B_D, B_P = (R_D + P - 1) // P, (R_P + P - 1) // P   # 20 / 40 blocks
S_D, S_P = B_D * P, B_P * P               # 2560 / 5120 slots per core
TBL_D = NC * S_D + 1                      # 20481; zero row at 20480
TBL_P = NC * S_P + 2                      # 40962; zero rows at 40960/40961
ZROW_D = NC * S_D                         # 20480
ZROW_P_HALF = NC * S_P // 2               # 20480 (row-pair index of both zero rows)

_CACHE = {}


# --------------------------------------------------------------------------
# host-side preprocessing
# --------------------------------------------------------------------------

def _prep_graph(src, dst, N, R, S, B, ppi):
    """Degree-sort nodes per core, build padded k-major gather indices.

    Returns dict with per-core: perm, operm [128,B] int32, and int16
    idx arrays (wrap-16 layout) plus per-block K lists (uniform over cores).
    """
    src = np.asarray(src).astype(np.int64)
    dst = np.asarray(dst).astype(np.int64)
    deg = np.bincount(dst, minlength=N)
    gid = np.empty(N, np.int64)
    perms = []
    for c in range(NC):
        lo = c * R
        perm = np.argsort(-deg[lo:lo + R], kind="stable")
        perms.append(perm)
        gid[lo + perm] = c * S + np.arange(R)

    # per-core CSR in slot order
    core_of = dst // R
    slot = gid[dst] - core_of * S
    vals = gid[src]
    cores = []
    for c in range(NC):
        m = core_of == c
        s_e = slot[m]
        v_e = vals[m]
        order = np.argsort(s_e, kind="stable")
        s_e, v_e = s_e[order], v_e[order]
        counts = np.bincount(s_e, minlength=S).astype(np.int64)
        starts = np.zeros(S + 1, np.int64)
        np.cumsum(counts, out=starts[1:])
        if ppi:
            par = (v_e % 2).astype(np.int64)
            cores.append((starts, v_e // 2, par, counts))
        else:
            cores.append((starts, v_e, None, counts))

    # per-block K (max over cores)
    if not ppi:
        K = []
        for b in range(B):
            k = 1
            for c in range(NC):
                k = max(k, int(cores[c][3][b * P:(b + 1) * P].max()))
            K.append(k)
        KE = KO = None
    else:
        KE, KO = [], []
        for b in range(B):
            ke = ko = 1
            for c in range(NC):
                starts, vh, par, _ = cores[c]
                for p in range(P):
                    s = b * P + p
                    pe = par[starts[s]:starts[s + 1]]
                    ne = int((pe == 0).sum())
                    ke = max(ke, ne)
                    ko = max(ko, len(pe) - ne)
            KE.append(ke)
            KO.append(ko)
        K = None

    def wrap16(seq):
        # seq [n], n % 16 == 0 -> [16, n/16] with [p, s] = seq[s*16+p]
        return seq.reshape(-1, 16).T.astype(np.int16)

    out = {"perms": perms, "K": K, "KE": KE, "KO": KO, "gid": gid}
    idxs, idxs_ev, idxs_od, operms = [], [], [], []
    for c in range(NC):
        starts, vh, par, _ = cores[c]
        perm = perms[c]
        op = np.full((P, B), R, np.int32)
        cols, cols_ev, cols_od = [], [], []
        for b in range(B):
            for p in range(P):
                s = b * P + p
                if s < R:
                    op[p, b] = perm[s]
            if not ppi:
                ids = np.full((K[b], P), ZROW_D, np.int64)
                for p in range(P):
                    s = b * P + p
                    seg = vh[starts[s]:starts[s + 1]]
                    ids[:len(seg), p] = seg
                cols.append(wrap16(ids.ravel()))
            else:
                ide = np.full((KE[b], P), ZROW_P_HALF, np.int64)
                ido = np.full((KO[b], P), ZROW_P_HALF, np.int64)
                for p in range(P):
                    s = b * P + p
                    seg = vh[starts[s]:starts[s + 1]]
                    pe = par[starts[s]:starts[s + 1]]
                    se = seg[pe == 0]
                    so = seg[pe == 1]
                    ide[:len(se), p] = se
                    ido[:len(so), p] = so
                cols_ev.append(wrap16(ide.ravel()))
                cols_od.append(wrap16(ido.ravel()))
        if not ppi:
            idxs.append(np.tile(np.concatenate(cols, axis=1), (8, 1)))
        else:
            idxs_ev.append(np.tile(np.concatenate(cols_ev, axis=1), (8, 1)))
            idxs_od.append(np.tile(np.concatenate(cols_od, axis=1), (8, 1)))
        operms.append(op)
    out["idx"] = idxs
    out["idx_ev"] = idxs_ev
    out["idx_od"] = idxs_od
    out["operm"] = operms
    return out


def _perm_transpose(h, perm, R, S):
    hp = np.zeros((S, D), np.float32)
    hp[:R] = np.asarray(h, np.float32)[perm]
    return np.ascontiguousarray(hp.T)


# --------------------------------------------------------------------------
# device program
# --------------------------------------------------------------------------

def _build(K_d, KE_p, KO_p, CD, CPE, CPO, rep=1):
    nc = bacc.Bacc("TRN2", target_bir_lowering=False, debug=False,
                   num_devices=NC, num_swdge_queues=4)
    f32, i16, i32 = mybir.dt.float32, mybir.dt.int16, mybir.dt.int32

    hT_d = nc.dram_tensor("hT_d", [P, S_D], f32, kind="ExternalInput")
    hT_p = nc.dram_tensor("hT_p", [P, S_P], f32, kind="ExternalInput")
    w_in = {}
    for g in ("d", "p"):
        for w in ("wpt", "wst", "wnt"):
            w_in[f"{w}_{g}"] = nc.dram_tensor(f"{w}_{g}", [P, D], f32, kind="ExternalInput")
        for b in ("bp", "bb"):
            w_in[f"{b}_{g}"] = nc.dram_tensor(f"{b}_{g}", [1, D], f32, kind="ExternalInput")
    idx_d = nc.dram_tensor("idx_d", [P, CD], i16, kind="ExternalInput")
    idx_pe = nc.dram_tensor("idx_pe", [P, CPE], i16, kind="ExternalInput")
    idx_po = nc.dram_tensor("idx_po", [P, CPO], i16, kind="ExternalInput")
    op_d = nc.dram_tensor("op_d", [P, B_D], i32, kind="ExternalInput")
    op_p = nc.dram_tensor("op_p", [P, B_P], i32, kind="ExternalInput")
    outs = {
        ("d", 0): nc.dram_tensor("od1", [R_D + 1, D], f32, kind="ExternalOutput"),
        ("p", 0): nc.dram_tensor("op1", [R_P + 1, D], f32, kind="ExternalOutput"),
        ("d", 1): nc.dram_tensor("od2", [R_D + 1, D], f32, kind="ExternalOutput"),
        ("p", 1): nc.dram_tensor("op2", [R_P + 1, D], f32, kind="ExternalOutput"),
    }

    G = {
        "d": dict(R=R_D, S=S_D, B=B_D, tbl_rows=TBL_D, K=K_d, hT=hT_d,
                  idx=idx_d, op=op_d),
        "p": dict(R=R_P, S=S_P, B=B_P, tbl_rows=TBL_P, KE=KE_p, KO=KO_p,
                  hT=hT_p, idx_ev=idx_pe, idx_od=idx_po, op=op_p),
    }

    qn = [0]

    def next_q():
        q = qn[0] % 4
        qn[0] += 1
        return q

    with tile.TileContext(nc) as tc:
        with (
            tc.tile_pool(name="const", bufs=1) as constp,
            tc.tile_pool(name="idxp", bufs=1) as idxp,
            tc.tile_pool(name="hT_d_pool", bufs=B_D + 6) as hTd_pool,
            tc.tile_pool(name="hT_p_pool", bufs=B_P + 6) as hTp_pool,
            tc.tile_pool(name="gat_d", bufs=2) as gat_d,
            tc.tile_pool(name="gat_p", bufs=2) as gat_p,
            tc.tile_pool(name="work", bufs=4) as work,
            tc.tile_pool(name="mout", bufs=4) as mout,
            tc.tile_pool(name="psum_m", bufs=2, space="PSUM") as psum_m,
            tc.tile_pool(name="psum_o", bufs=2, space="PSUM") as psum_o,
            tc.tile_pool(name="psum_t", bufs=3, space="PSUM") as psum_t,
            tc.tile_pool(name="dram", bufs=1, space="DRAM") as dram,
        ):
            # constants
            ones = constp.tile([1, D], f32, tag="ones", name="ones")
            nc.vector.memset(ones[:], 1.0)
            zrow = constp.tile([1, D], f32, tag="zrow", name="zrow")
            nc.vector.memset(zrow[:], 0.0)
            ident = constp.tile([P, P], f32, tag="ident", name="ident")
            make_identity(nc, ident[:])
            wt = {}
            for name, t in w_in.items():
                w_tile = constp.tile(list(t.shape), f32, tag=name, name=f"w_{name}")
                nc.sync.dma_start(out=w_tile[:], in_=t[:])
                wt[name] = w_tile

            idx_tiles = {}
            for name, t in (("d", idx_d), ("pe", idx_pe), ("po", idx_po)):
                it = idxp.tile(list(t.shape), i16, tag=f"idx{name}", name=f"idxt_{name}")
                nc.sync.dma_start(out=it[:], in_=t[:])
                idx_tiles[name] = it
            op_tiles = {}
            for name, t in (("d", op_d), ("p", op_p)):
                it = idxp.tile(list(t.shape), i32, tag=f"op{name}", name=f"opt_{name}")
                nc.sync.dma_start(out=it[:], in_=t[:])
                op_tiles[name] = it

            # DRAM tables + shard bounces (reused across hops)
            tbl = {g: dram.tile([G[g]["tbl_rows"], D], f32, tag=f"tbl_{g}", name=f"tbl_{g}")
                   for g in ("d", "p")}
            msh = {g: dram.tile([G[g]["S"], D], f32, tag=f"msh_{g}", name=f"msh_{g}")
                   for g in ("d", "p")}
            nc.sync.dma_start(out=tbl["d"][ZROW_D:ZROW_D + 1, :], in_=zrow[:])
            nc.sync.dma_start(out=tbl["p"][NC * S_P:NC * S_P + 1, :], in_=zrow[:])
            nc.sync.dma_start(out=tbl["p"][NC * S_P + 1:NC * S_P + 2, :], in_=zrow[:])

            hT_cur = {"d": [None] * B_D, "p": [None] * B_P}
            hT_nxt = {"d": [None] * B_D, "p": [None] * B_P}
            pools_hT = {"d": hTd_pool, "p": hTp_pool}
            gat_pools = {"d": gat_d, "p": gat_p}

            def phase_a(g, hop):
                gg = G[g]
                for b in range(gg["B"]):
                    if hop == 0:
                        hTb = pools_hT[g].tile([P, P], f32, tag=f"hT_{g}", name=f"hTb_{g}")
                        nc.sync.dma_start(out=hTb[:],
                                          in_=gg["hT"][:, b * P:(b + 1) * P])
                        hT_cur[g][b] = hTb
                    psm = psum_m.tile([P, P], f32, tag="psm", name="psm")
                    nc.tensor.matmul(psm[:], lhsT=hT_cur[g][b][:],
                                     rhs=wt[f"wpt_{g}"][:], start=True, stop=False)
                    nc.tensor.matmul(psm[:], lhsT=ones[:], rhs=wt[f"bp_{g}"][:],
                                     start=False, stop=True)
                    msb = mout.tile([P, P], f32, tag="m", name="msb")
                    nc.scalar.activation(msb[:], psm[:],
                                         mybir.ActivationFunctionType.Relu)
                    nc.sync.dma_start(out=msh[g][b * P:(b + 1) * P, :], in_=msb[:])
                nc.gpsimd.collective_compute(
                    "AllGather", mybir.AluOpType.bypass,
                    replica_groups=[list(range(NC))],
                    ins=[msh[g][:].opt()],
                    outs=[tbl[g][0:NC * gg["S"], :].opt()],
                )

            def phase_b(g, hop):
                gg = G[g]

                def do_gather(gt, out_koff, idx_tile, idx_koff, kcols, src_ap,
                              elem_step=None):
                    num = P * kcols
                    nc.gpsimd.dma_gather(
                        gt[:, out_koff * P:(out_koff + kcols) * P]
                            .rearrange("p (k f) -> p k f", f=D),
                        src_ap,
                        idx_tile[:, idx_koff * 8:(idx_koff + kcols) * 8],
                        num, num, D,
                        elem_step=elem_step,
                        single_packet=False,
                        queue_num=next_q(),
                    )

                for b in range(gg["B"]):
                    if g == "d":
                        kb = gg["K"][b]
                        ktot = kb
                    else:
                        ke, ko = gg["KE"][b], gg["KO"][b]
                        ktot = ke + ko
                    gather_tile = gat_pools[g].tile([P, ktot * P], f32, tag=f"g_{g}", name=f"gt_{g}")
                    if g == "d":
                        do_gather(gather_tile, 0, idx_tiles["d"], sum(gg["K"][:b]),
                                  kb, tbl["d"][:])
                    else:
                        pair_view = tbl["p"][:].rearrange("(n two) f -> n (two f)", two=2)
                        do_gather(gather_tile, 0, idx_tiles["pe"], sum(gg["KE"][:b]),
                                  ke, pair_view[:, 0:D], elem_step=2 * D)
                        do_gather(gather_tile, ke, idx_tiles["po"], sum(gg["KO"][:b]),
                                  ko, pair_view[:, D:2 * D], elem_step=2 * D)
                    agg = work.tile([P, P], f32, tag="agg", name="agg")
                    nc.vector.tensor_reduce(
                        out=agg[:],
                        in_=gather_tile[:].rearrange("p (k f) -> p f k", f=D),
                        axis=mybir.AxisListType.X, op=mybir.AluOpType.max,
                    )
                    ps_t = psum_t.tile([P, P], f32, tag="pst", name="ps_t")
                    nc.tensor.transpose(ps_t[:], agg[:], ident[:])
                    aggT = work.tile([P, P], f32, tag="aggT", name="aggT")
                    nc.vector.tensor_copy(out=aggT[:], in_=ps_t[:])
                    pso = psum_o.tile([P, P], f32, tag="pso", name="pso")
                    nc.tensor.matmul(pso[:], lhsT=hT_cur[g][b][:],
                                     rhs=wt[f"wst_{g}"][:], start=True, stop=False)
                    nc.tensor.matmul(pso[:], lhsT=aggT[:], rhs=wt[f"wnt_{g}"][:],
                                     start=False, stop=False)
                    nc.tensor.matmul(pso[:], lhsT=ones[:], rhs=wt[f"bb_{g}"][:],
                                     start=False, stop=True)
                    osb = mout.tile([P, P], f32, tag="osb", name="osb")
                    nc.scalar.activation(osb[:], pso[:],
                                         mybir.ActivationFunctionType.Relu)
                    nc.gpsimd.indirect_dma_start(
                        out=outs[(g, hop)][:],
                        out_offset=bass.IndirectOffsetOnAxis(
                            ap=op_tiles[g][:, b:b + 1], axis=0),
                        in_=osb[:],
                        in_offset=None,
                    )
                    if hop == 0:
                        ps_t2 = psum_t.tile([P, P], f32, tag="pst", name="ps_t2")
                        nc.tensor.transpose(ps_t2[:], osb[:], ident[:])
                        hT1 = pools_hT[g].tile([P, P], f32, tag=f"hT_{g}", name=f"hT1_{g}")
                        nc.vector.tensor_copy(out=hT1[:], in_=ps_t2[:])
                        hT_nxt[g][b] = hT1

            for _rep in range(rep):
                for hop in (0, 1):
                    phase_a("d", hop)
                    phase_a("p", hop)
                    phase_b("d", hop)
                    phase_b("p", hop)
                    if hop == 0:
                        hT_cur, hT_nxt = hT_nxt, {"d": [None] * B_D, "p": [None] * B_P}
                hT_cur = {"d": [None] * B_D, "p": [None] * B_P}
                hT_nxt = {"d": [None] * B_D, "p": [None] * B_P}

    nc.compile()
    return nc


# --------------------------------------------------------------------------
# entry point
# --------------------------------------------------------------------------

def kernel(h_dr, h_p, ddi_src, ddi_dst, ppi_src, ppi_dst,
           Wp_d, bp_d, Ws_d, Wn_d, b_d,
           Wp_p, bp_p, Ws_p, Wn_p, b_p):
    h_dr = np.asarray(h_dr, np.float32)
    h_p = np.asarray(h_p, np.float32)

    pd = _prep_graph(ddi_src, ddi_dst, N_DR, R_D, S_D, B_D, ppi=False)
    pp = _prep_graph(ppi_src, ppi_dst, N_P, R_P, S_P, B_P, ppi=True)

    CD = pd["idx"][0].shape[1]
    CPE = pp["idx_ev"][0].shape[1]
    CPO = pp["idx_od"][0].shape[1]
    key = (tuple(pd["K"]), tuple(pp["KE"]), tuple(pp["KO"]))
    if key not in _CACHE:
        _CACHE[key] = _build(pd["K"], pp["KE"], pp["KO"], CD, CPE, CPO)
    nc = _CACHE[key]

    wts = {
        "wpt_d": np.ascontiguousarray(np.asarray(Wp_d, np.float32).T),
        "wst_d": np.ascontiguousarray(np.asarray(Ws_d, np.float32).T),
        "wnt_d": np.ascontiguousarray(np.asarray(Wn_d, np.float32).T),
        "bp_d": np.asarray(bp_d, np.float32).reshape(1, D),
        "bb_d": np.asarray(b_d, np.float32).reshape(1, D),
        "wpt_p": np.ascontiguousarray(np.asarray(Wp_p, np.float32).T),
        "wst_p": np.ascontiguousarray(np.asarray(Ws_p, np.float32).T),
        "wnt_p": np.ascontiguousarray(np.asarray(Wn_p, np.float32).T),
        "bp_p": np.asarray(bp_p, np.float32).reshape(1, D),
        "bb_p": np.asarray(b_p, np.float32).reshape(1, D),
    }

    in_maps = []
    for c in range(NC):
        m = dict(wts)
        m["hT_d"] = _perm_transpose(h_dr[c * R_D:(c + 1) * R_D], pd["perms"][c], R_D, S_D)
        m["hT_p"] = _perm_transpose(h_p[c * R_P:(c + 1) * R_P], pp["perms"][c], R_P, S_P)
        m["idx_d"] = pd["idx"][c]
        m["idx_pe"] = pp["idx_ev"][c]
        m["idx_po"] = pp["idx_od"][c]
        m["op_d"] = pd["operm"][c]
        m["op_p"] = pp["operm"][c]
        in_maps.append(m)

    res = bass_utils.run_bass_kernel_spmd(nc, in_maps, core_ids=list(range(NC)))

    h_dr1 = np.concatenate([res.results[c]["od1"][:R_D] for c in range(NC)], axis=0)
    h_p1 = np.concatenate([res.results[c]["op1"][:R_P] for c in range(NC)], axis=0)
    h_dr2 = np.concatenate([res.results[c]["od2"][:R_D] for c in range(NC)], axis=0)
    h_p2 = np.concatenate([res.results[c]["op2"][:R_P] for c in range(NC)], axis=0)
    return (h_dr1, h_p1, h_dr2, h_p2)
